# revision 19
# baseline (speedup 1.0000x reference)
"""Trainium2 Bass kernel for nn_ModelNew_3556232922178 (dense_cnn).

Reference computation (B=16, Cin=32, D=H=W=32, Cout=64, k=3):
    y = ConvTranspose3d(x, W, stride=1, pad=0)      # full correlation, out 34^3
    y = (y + bias) * SCALE
    y = (y - running_mean) * rsqrt(running_var+EPS)  # inference BN
    out = y.mean over spatial                        # (B, Cout)

Because the global average pool sums over the *entire* full-correlation
output, every (input voxel, kernel tap) product contributes exactly once:
    sum_spatial(conv)[b,o] = sum_i (sum_spatial x)[b,i] * (sum_taps W)[o,i]
so the whole network collapses to a per-(b,i) spatial reduction of x, a
(B,Cin)x(Cin,Cout) matmul, and a per-channel affine. The affine is folded
entirely on the host:
    w2[(i,q), o] = (sum_taps W)[o,i] * SCALE/34^3 * rsqrt(rv[o]+EPS)
    bb[b, o]     = (bias[o]*SCALE - rm[o]) * rsqrt(rv[o]+EPS)
    out[b,o]     = sum_iq red[(i,q), b] * w2[(i,q), o] + bb[b,o]

Sharding: data-parallel over batch, 2 batches per core, 8 cores. Each core
reduces its own x shard (8.4 MB — the dominant, DMA-bound cost at the
~420 GB/s per-core HBM share), computes its two output rows completely,
no collectives. Host concatenates.

Device schedule per core (trace-driven; ~6.8 us of NEFF/NRT preamble and
~6.7 us of NRT epilogue — the runtime's full-semaphore-file reset wall +
final barrier — bracket everything and are fixed):
  x viewed as (2, 128, 8192): partition p = i*4 + q over (channel i,
  spatial quarter q). ALL x chunks ride the single SP HWDGE queue
  (measured: a second queue adds ~3 us start skew and no aggregate BW),
  4 chunks per batch (8 DMAs = Tile's proc count, no proc-wrap waits),
  batches interleaved chunk-wise, sizes descending 4096/2048/1024/1024
  (16 KB head descriptors burst fastest; nothing below 4 KB). Triggers
  are hoisted post-build to the head of the entry preamble block.
  Batch 1 chunks (queue lead) reduce on ACT (activation Copy +
  accum_out), batch 0 on DVE. Chunk columns k0..k2 of each batch are
  row-summed into red[(iq), b] as soon as available; an early K=128
  accumulating PE matmul (stationary=red (128,2), moving=w2 (128,64) →
  psum (2,64)) folds them before the last chunks land. The two last-chunk
  columns go to last2: DVE reduces b0k3 into last2[:,0] and then copies
  ACT's b1k3 accumulator column into last2[:,1] (the copy both launders
  the cross-engine dependency into a single DVE tick and orders the late
  matmul behind both columns with ONE sem wait — the walrus build rejects
  instructions with >1 wait). The late matmul accumulates last2 into the
  same psum; DVE adds the host-folded bias row (psum + bb) into SBUF, and
  SP stores the (2,64) result with a 2-descriptor DMA.

  There is NO kernel exit machinery: no drain chain, no semaphore clear,
  no exit gate. The NRT wrapper that brackets every NEFF execution resets
  the ENTIRE semaphore file (S[3..255], split across engines) after the
  kernel block, so Tile's own clear_and_free is pure duplication; each
  engine's last kernel instruction falls straight into the wrapper's exit
  barrier, which starts the (fixed ~6.2 us) reset wall that much earlier.
  The y store is emitted raw (untracked) with a single DVE sem wait — all
  other outstanding work is transitively ordered before DVE's final tick.
  Nothing waits on the store's completion: it lands during the wrapper
  reset wall. Re-executability is verified by the harness's 50-iteration
  re-execution check.
"""

import numpy as np

import concourse.bass as bass
from concourse import mybir
from concourse.tile import TileContext
from concourse.vector_clock import ScopedClock
from concourse.bass_utils import run_bass_kernel_spmd

EPS = 1e-5
SCALE = 2.0
B, CIN, S = 16, 32, 32 * 32 * 32
COUT, KT = 64, 27
NCORES = 8
BPC = B // NCORES          # batches per core
Q = 4                      # spatial quarters -> 128 partitions
F = S // Q                 # 8192 elements per partition per batch
NSPATIAL = 34 * 34 * 34    # conv output positions (pool divisor)
# free-axis chunk sizes per batch. Descriptors are one partition-row of a
# chunk (size*4 bytes). Measured: 16 KB descriptors on a single queue
# sustain ~460 GB/s; <=2 KB descriptors collapse to ~40 GB/s. So ALL
# chunks ride ONE HWDGE queue, batches interleaved chunk-wise, sizes
# descending and none below 1024 (4 KB).
CHUNKS = [4096, 2048, 1024, 1024]
assert sum(CHUNKS) == F
NCH = len(CHUNKS)
F32 = mybir.dt.float32

# If True, post-build move all x DMA triggers into the framework
# preamble block, before each engine's entry-barrier arrive — the DMA
# window starts ~1.5 us earlier, overlapping barrier+branch overhead.
HOIST_TRIGGERS = True
# If True, restore a conservative Tile exit (drain chain + sem clear +
# barriers) instead of falling straight into the NRT wrapper epilogue.
SAFE_EXIT = False

TRACE = False              # set by test harness to collect an NTFF profile
LAST_RESULT = None         # BassKernelResults of the most recent run


class LeanExitTileContext(TileContext):
    """TileContext whose exit emits ONLY the raw y store (no drains, no
    semaphore clear, no barriers).

    The NRT wrapper around every NEFF execution resets the entire
    semaphore file after the kernel block, so Tile's clear_and_free (and
    the drain chain that orders it) duplicates work the runtime does
    anyway — at ~1.3 us of critical-path cost (the reset wall can't start
    until every engine arrives at the wrapper's exit barrier, and the
    drain/clear chain delays the slowest arrival).

    The y store is emitted here so it lands at the tail of SP's stream
    with a single DVE sem wait (out_s is DVE's final tick; every other
    outstanding proc is transitively ordered before it — chunk DMA sems
    are consumed by the reduces, DMASW const sems by the DVE copies, the
    ACT tick by the acol copy, the PE tick by the bias add).
    """

    def __init__(self, nc, store_args):
        super().__init__(nc)
        self._store_args = store_args

    def _single_dve_wait(self, inst, tick_clock, wait_clock):
        """Attach this kernel's full outstanding-wait set to `inst`, then
        prune it to the single DVE wait (the final tick; everything else
        is transitively ordered before it)."""
        wait_clock.add_sem_waits(
            inst.ins, ScopedClock({None: tick_clock.global_clock})
        )
        si = inst.ins.sync_info
        waits = list(si.on_wait) if si is not None and si.on_wait else []
        updates = list(si.on_update) if si is not None and si.on_update else []
        kept = [w for w in waits if (w.ant_name or "").startswith("DVE")]
        assert len(kept) == 1, [w.ant_name for w in waits]
        inst.ins.sync_info = mybir.SyncInfo(on_wait=kept, on_update=updates)

    def _drain_and_barrier(self, tick_clock, wait_clock):
        if SAFE_EXIT:
            return super()._drain_and_barrier(tick_clock, wait_clock)
        y, out_s, y_sem = self._store_args[:3]
        store = self.nc.sync.dma_start(out=y[:, :], in_=out_s).then_inc(y_sem, 16)
        self._single_dve_wait(store, tick_clock, wait_clock)
        # No drains, no sem clear, no barriers: the NRT wrapper's reset
        # wall zeroes the whole semaphore file after the kernel block.
        # The one piece of Tile's exit the wrapper does NOT duplicate —
        # HWDGE ring-state reset for the kernel's DMA sems (without it,
        # ring bookkeeping goes stale across re-executions: trigger
        # pushes stall and small-descriptor bandwidth drops ~30%) — is
        # moved to the ENTRY preamble of the NEXT execution (see
        # _build_program), where it is completely off the critical path.
        # It must not race an active push, so at exit we only record the
        # sem range for the entry-side drain to use.
        assert self.sems is not None
        nums = sorted(h.num for h in self.sems.allocated().values())
        nums = sorted(set(nums) | {y_sem.num})
        assert nums == list(range(nums[0], nums[0] + len(nums))), nums
        self._store_args[3] = range(nums[0], nums[-1] + 1)
        popped = self.nc._tile_sem_poison_stack.pop()
        assert popped is self._sem_poison


def _build_program():
    nc = bass.Bass()
    x = nc.dram_tensor("x", (BPC, 128, F), F32, kind="ExternalInput")
    # Host-prepared tap-reduced W^T replicated over the 4 quarter groups
    # and pre-scaled by the folded BN multiplier:
    # w2[(i*4+q), o] = sum_t weight[o, i, t] * SCALE/34^3 * rsqrt(rv[o]+EPS)
    w2 = nc.dram_tensor("w2", (128, COUT), F32, kind="ExternalInput")
    # Host-folded BN bias column:
    # bb[o] = (bias[o]*SCALE - rm[o]) * rsqrt(rv[o]+EPS)
    bb = nc.dram_tensor("bb", (COUT, 1), F32, kind="ExternalInput")
    y = nc.dram_tensor("y", (COUT, BPC), F32, kind="ExternalOutput")

    # Completion sem for the untracked y store (walrus rejects a DGE with
    # no sync info). Nothing waits on it; the NRT wrapper's reset wall
    # zeroes it each run.
    y_sem = nc.alloc_semaphore("y_store_sem")

    store_box = [None, None, y_sem, None]
    with LeanExitTileContext(nc, store_box) as tc:
        with (
            tc.tile_pool(name="const", bufs=1) as const,
            tc.tile_pool(name="xbuf", bufs=1) as xbuf,
            tc.tile_pool(name="ps", bufs=1, space="PSUM") as ps,
        ):
            # All 8 x chunk triggers on the SP queue, batches interleaved
            # chunk-wise (b1 leads: ACT reduces b1 and carries the extra
            # accumulator-read overhead, so it gets the earliest data).
            # 8 DMAs = 8 Tile procs, so no trigger carries a proc-wrap
            # wait, and all are hoisted into the entry preamble.
            xts = {}
            hoistable = []
            for k, sz in enumerate(CHUNKS):
                start = sum(CHUNKS[:k])
                for b in (1, 0):
                    xt = xbuf.tile([128, sz], F32, name=f"x{b}_{k}", tag=f"x{b}_{k}")
                    trig = nc.sync.dma_start(out=xt, in_=x[b, :, start : start + sz])
                    hoistable.append(trig.ins)
                    xts[(b, k)] = xt

            # Folded consts — tiny, via SWDGE (q0), overlapped with the x
            # window. NOT hoisted: Pool leads the entry barrier and its
            # barrier drain waits for Pool's outstanding DMA data.
            w2t = const.tile([128, COUT], F32)
            nc.gpsimd.dma_start(out=w2t, in_=w2[:, :])
            bbt = const.tile([COUT, 1], F32)
            nc.gpsimd.dma_start(out=bbt, in_=bb[:, :])

            # DVE-side copies of the small SWDGE inputs so matmul/add
            # operands are DVE-produced and carry a single sem wait. DVE
            # is idle until the first big chunk lands, so these are free.
            w2_s = const.tile([128, COUT], F32)
            bb_s = const.tile([COUT, 1], F32)
            nc.vector.tensor_copy(w2_s, w2t)
            nc.vector.tensor_copy(bb_s, bbt)

            # Dummy activation on the framework zero-constant, hoisted
            # into ACT's preamble: pulls the ~1.3 us activation-table
            # load into the entry preamble, far ahead of the first
            # chunk's arrival.
            warm = const.tile([128, 1], F32)
            hoistable.append(
                nc.scalar.activation(
                    out=warm,
                    in_=nc.const_aps.aps[(F32, 0.0)],
                    func=mybir.ActivationFunctionType.Copy,
                ).ins
            )

            # Spatial reduction. Batch 1 chunks on ACT (activation Copy +
            # accum_out), batch 0 on DVE. Chunks k0..k2 go to the stats
            # tiles (folded early); the k3 columns go to kcol/acol.
            stats_d = const.tile([128, NCH - 1], F32)
            stats_a = const.tile([128, NCH - 1], F32)
            kcol = const.tile([128, 1], F32)
            acol = const.tile([128, 1], F32)

            # ACT: b1 chunks in landing order.
            for k, sz in enumerate(CHUNKS):
                scratch = const.tile([128, sz], F32, name=f"scr{k}", tag=f"scr{k}")
                nc.scalar.activation(
                    out=scratch,
                    in_=xts[(1, k)],
                    func=mybir.ActivationFunctionType.Copy,
                    accum_out=(
                        stats_a[:, k : k + 1] if k < NCH - 1 else acol[:, 0:1]
                    ),
                )

            # DVE: b0 chunks k0..k2, then the two row-sums, then the b0k3
            # reduce, then the acol copy — this order keeps the early
            # matmul's DVE-tick wait below the last-chunk work, and the
            # late matmul's single DVE wait above BOTH last2 columns.
            red = const.tile([128, BPC], F32)
            for k in range(NCH - 1):
                nc.vector.reduce_sum(
                    out=stats_d[:, k : k + 1],
                    in_=xts[(0, k)],
                    axis=mybir.AxisListType.X,
                )
            nc.vector.reduce_sum(
                out=red[:, 0:1], in_=stats_d[:, :], axis=mybir.AxisListType.X
            )
            nc.vector.reduce_sum(
                out=red[:, 1:2], in_=stats_a[:, :], axis=mybir.AxisListType.X
            )
            nc.vector.reduce_sum(
                out=kcol[:, 0:1], in_=xts[(0, NCH - 1)], axis=mybir.AxisListType.X
            )

            # psum[o, b] = sum_iq w2[(iq), o] * red[(iq), b] (+ k3 cols):
            # the early matmul folds chunks k0..k2 of both batches before
            # the last chunks land; two single-column late matmuls fold
            # the k3 columns, each gated by exactly one producer engine
            # (lateA fires as soon as ACT's accumulator read lands, while
            # DVE is still reducing b0k3 — no cross-engine laundering op
            # on the critical path). Column slices of PSUM keep the
            # partition base at 0, which sub-region matmuls require.
            pm = ps.tile([COUT, BPC], F32)
            nc.tensor.matmul(pm, w2_s, red, start=True, stop=False)
            nc.tensor.matmul(
                pm[:, 1:2], w2_s, acol, start=False, stop=False,
                skip_group_check=True,
            )
            nc.tensor.matmul(
                pm[:, 0:1], w2_s, kcol, start=False, stop=True,
                skip_group_check=True,
            )

            # Bias add (the whole BN affine, host-folded) straight off
            # PSUM; this is DVE's final tick and the store's one wait.
            out_s = const.tile([COUT, BPC], F32)
            nc.vector.tensor_scalar(
                out=out_s,
                in0=pm,
                scalar1=bb_s[:, 0:1],
                scalar2=None,
                op0=mybir.AluOpType.add,
            )
            store_box[0] = y
            store_box[1] = out_s

    # Entry-side DGE ring-state reset for the previous execution's DMA
    # sems (see LeanExitTileContext): emitted after the Tile context (the
    # sem range is only known then) and hoisted to the HEAD of the entry
    # preamble, in SP program order BEFORE the first chunk trigger. At
    # that point the previous execution is fully quiesced (its store
    # landed during its own epilogue wall, >30 us plus a host round trip
    # ago), so the reset races nothing — unlike an exit-side reset, which
    # either races the store push (corrupting the ring) or must trail it
    # (delaying the wrapper's reset wall).
    entry_reset = nc.sync.drain(semaphore_range=store_box[3])
    hoistable.insert(0, entry_reset.ins)

    _elide_redundant_waits(nc)
    if HOIST_TRIGGERS:
        _hoist_triggers_to_preamble(nc, hoistable)
    return nc


def _elide_redundant_waits(nc):
    """Drop sem waits that are implied by engine program order.

    Two safe elisions, applied per engine instruction stream (engines
    execute in order):
      1. A wait on the instruction's OWN engine's tick sem — the producer
         precedes it in the same stream, so the dep holds by in-order
         execution alone.
      2. A wait dominated by an earlier instruction of the same stream
         that already waited on the same sem for >= the value.
    Tile emits both kinds (e.g. a DVE self-wait on the k3 reduce feeding
    the next DVE op; a low-valued DVE wait on a PE matmul whose earlier
    matmul already waited a higher DVE tick), and the walrus codegen
    rejects instructions carrying more than one wait. Assert every
    instruction ends up with <= 1 wait.
    """
    seen = {}  # engine -> {sem_name: max waited value}
    for f in nc.m.functions:
        for bb in f.blocks:
            for inst in bb.instructions:
                si = inst.sync_info
                if si is None or not si.on_wait:
                    continue
                eng = getattr(inst, "engine", None)
                eng_key = getattr(eng, "value", eng)
                prefix = f"{eng_key}_"
                waited = seen.setdefault(eng_key, {})
                kept = []
                for w in si.on_wait:
                    name = w.ant_name or ""
                    if name.startswith(prefix):
                        continue  # self-engine: program order implies it
                    if waited.get(name, -1) >= w.wait_value:
                        continue  # dominated by an earlier wait
                    kept.append(w)
                for w in kept:
                    name = w.ant_name or ""
                    if w.wait_value > waited.get(name, -1):
                        waited[name] = w.wait_value
                assert len(kept) <= 1, (
                    inst.name,
                    prefix,
                    [(w.ant_name, w.wait_value) for w in si.on_wait],
                )
                if len(kept) != len(si.on_wait):
                    inst.sync_info = mybir.SyncInfo(
                        on_wait=kept, on_update=list(si.on_update or [])
                    )


def _hoist_triggers_to_preamble(nc, trigger_insts):
    """Move wait-free DMA triggers into the entry-preamble block.

    Each engine's kernel code sits behind the framework preamble's
    all-engine barrier + branch (~1.5 us of entry latency after register
    init). The x-chunk loads have no dependencies at all, so their
    triggers are moved to the head of the preamble block — the DMA
    window opens that much earlier.
    """
    f = nc.m.functions[0]
    pre, kernel_bbs = f.blocks[0], f.blocks[1:]
    moved = 0
    for inst in trigger_insts:
        si = inst.sync_info
        assert not (si and si.on_wait), f"hoistable trigger has waits: {inst.name}"
        src_bb = None
        for bb in kernel_bbs:
            if inst in bb.instructions:
                src_bb = bb
                break
        assert src_bb is not None, f"trigger {inst.name} not found"
        src_bb.instructions.remove(inst)
        pre.instructions.insert(moved, inst)
        moved += 1
    assert moved == len(trigger_insts)


def prep_inputs(x, weight, bias, running_mean, running_var):
    """Host-side sharding prep: per-core in_maps for run_bass_kernel_spmd."""
    x = np.ascontiguousarray(np.asarray(x, dtype=np.float32))
    weight = np.ascontiguousarray(np.asarray(weight, dtype=np.float32))
    bias = np.asarray(bias, dtype=np.float32)
    rm = np.asarray(running_mean, dtype=np.float32)
    rv = np.asarray(running_var, dtype=np.float32)

    xv = x.reshape(B, 128, F)          # (b, i*4+q, f) — contiguous view
    rstd = (1.0 / np.sqrt(rv + np.float32(EPS))).astype(np.float32)
    alpha = (np.float32(SCALE / NSPATIAL) * rstd).astype(np.float32)
    beta = ((bias * np.float32(SCALE) - rm) * rstd).astype(np.float32)
    # Tap-reduce W, replicate W^T across the 4 quarter groups, fold alpha.
    w2 = np.ascontiguousarray(
        np.repeat(weight.reshape(COUT, CIN, KT).sum(axis=2).T, Q, axis=0)
        * alpha[None, :]
    ).astype(np.float32)
    bb = np.ascontiguousarray(beta[:, None]).astype(np.float32)
    return [
        {"x": xv[k * BPC : (k + 1) * BPC], "w2": w2, "bb": bb}
        for k in range(NCORES)
    ]


def kernel(x, weight, bias, running_mean, running_var):
    global LAST_RESULT
    in_maps = prep_inputs(x, weight, bias, running_mean, running_var)
    nc = _build_program()
    res = run_bass_kernel_spmd(
        nc, in_maps, core_ids=list(range(NCORES)), trace=TRACE
    )
    LAST_RESULT = res

    out = np.empty((B, COUT), dtype=np.float32)
    for k in range(NCORES):
        out[k * BPC : (k + 1) * BPC] = res.results[k]["y"].T
    return out
